# revision 28
# baseline (speedup 1.0000x reference)
"""Local window attention (7x7 windows, 8 heads, d=64) Trainium2 Bass kernel.

Full inputs in, full outputs out. Data-parallel over batch across 8 cores
(4 images per core). Shapes hardcoded per spec:
  fmap (32, 56, 56, 256) f32, Wq (256,512), Wkv (256,1024), Wo (512,256), bo (256,)

Final design (2.88ms baseline -> 0.543ms, 5.3x). Key facts learned on HW:
engines execute their queues IN ORDER and the tile scheduler follows
emission order, so cross-group overlap must be software-pipelined
explicitly, with each engine's per-iteration queue sorted by dependency
readiness. fp32 matmuls are 4 cy/row vs bf16's 1. DMA triggers cost
~700-800ns on the issuing sequencer (spread across sync/gpsimd/scalar
queues). GPSIMD tensor ops are ~3x slower than DVE/ACT and cannot touch
PSUM. DVE reciprocal costs ~7.4ns per FREE element (keep free dims tiny).
Engine-op partition ranges must start 32-aligned; matmul operands must
start at partition 0 (high-half streaming bug).

Per group g = 2 adjacent-y windows, tokens 64-padded (p = 64w+7r+t); fmap
is cast to bf16 on the HOST (exec-time is NEFF time; halves input DMA).
Stages, emitted with skews P:+1 A:0 B:-1 C1:-2 C2:-3 D:-4 (6 groups in
flight, 9 PSUM bank allocs per iteration on the 8-bank rotation):

  P:  2 input DMAs ([49,256] <- [7,7,256]; gpsimd+sync queues)
  A:  2 bf16 PE transposes (N=113) -> fT (copy split ACT/DVE); q/k
      projections (16 mm, N=98 compact via strided rhs); qT2 one
      full-partition DVE copy; kT2 block-diag over 2 heads (2 ACT copies,
      zero quadrants persist in rotated buffers); v (2 mm) -> one
      full-partition ACT copy into v_raw (persistent ones column at 64)
  B:  ST 2-heads-per-matmul (8 mm, K=128 d-stack, N=49); exp: 4 ACT ops
      into a zero-padded arena expSz (w0 rows 0:49, w1 rows 64:113,
      zeros elsewhere kill junk in the K=113 av contraction)
  C1: av+denom (16 mm, K=113, N=65, lhsT=expSz, rhs=v_raw|1); tiny
      reciprocal of the denom column (free size 4!); normalize into
      out_tok via scalar_tensor_tensor with stride-0 broadcast_to of 1/den
  C2: 4 bf16 PE transposes (N=113) -> outT (DVE copy)
  D:  fin = outT.T @ Wo + bo (4 mm + DVE stt); 2 output DMAs (sync+gpsimd)

All matmuls bf16 (psum fp32); casts ride the mandatory psum->SBUF copies.
"""

from contextlib import ExitStack

import numpy as np

import concourse.bacc as bacc
import concourse.bass as bass
import concourse.tile as tile
from concourse import mybir
from concourse import bass_isa
from concourse.masks import make_identity
from concourse.bass_utils import run_bass_kernel_spmd

P = 7
PP = 49          # tokens per window
H = 8            # heads
D = 64           # head dim
DIM = 256        # channels
INNER = 512      # h*d
SCALE = D ** -0.5
IMGS_PER_CORE = 4
NCORES = 8
X = 56
NW = X // P      # 8 windows per axis
FP32 = mybir.dt.float32
BF16 = mybir.dt.bfloat16
NROT = 4         # manual rotation depth for persistent tiles


def build_bass(n_imgs=IMGS_PER_CORE):
    nc = bacc.Bacc("TRN2", target_bir_lowering=False, debug=False)

    fm = nc.dram_tensor("fmap", [n_imgs, X, X, DIM], BF16, kind="ExternalInput").ap()
    wq = nc.dram_tensor("Wq", [DIM, INNER], FP32, kind="ExternalInput").ap()
    wkv = nc.dram_tensor("Wkv", [DIM, 2 * INNER], FP32, kind="ExternalInput").ap()
    wo = nc.dram_tensor("Wo", [INNER, DIM], FP32, kind="ExternalInput").ap()
    bo = nc.dram_tensor("bo", [DIM], FP32, kind="ExternalInput").ap()
    out = nc.dram_tensor("out", [n_imgs, X, X, DIM], FP32, kind="ExternalOutput").ap()

    with tile.TileContext(nc) as tc:
        with ExitStack() as ctx:
            build_kernel(ctx, tc, out, fm, wq, wkv, wo, bo, n_imgs)
    nc.compile()
    return nc


def build_kernel(ctx, tc, out, fm, wq, wkv, wo, bo, n_imgs=IMGS_PER_CORE):
    nc = tc.nc
    consts = ctx.enter_context(tc.tile_pool(name="consts", bufs=1))
    sb = ctx.enter_context(tc.tile_pool(name="sb", bufs=6))
    ps = ctx.enter_context(tc.tile_pool(name="ps", bufs=8, space="PSUM"))

    # ---- constants ----
    ident = consts.tile([128, 128], FP32)
    make_identity(nc, ident[:])

    ones = consts.tile([1, 128], FP32)
    nc.gpsimd.memset(ones[:], 1.0)
    identb = consts.tile([128, 128], BF16)
    nc.vector.tensor_copy(identb[:], ident[:])

    def stage_w(dram_ap, shape, name):
        st = sb.tile(shape, FP32, tag="stage")
        nc.sync.dma_start(out=st[:], in_=dram_ap)
        bt = consts.tile(shape, BF16, tag=name)
        nc.vector.tensor_copy(bt[:], st[:])
        return bt

    wq_s = stage_w(wq.rearrange("(kc ck) n -> ck kc n", ck=128), [128, 2, INNER],
                   "wq_s")
    wk_s = stage_w(wkv[:, 0:INNER].rearrange("(kc ck) n -> ck kc n", ck=128),
                   [128, 2, INNER], "wk_s")
    wv_s = stage_w(wkv[:, INNER:2 * INNER].rearrange("(kc ck) n -> ck kc n", ck=128),
                   [128, 2, INNER], "wv_s")
    wo_s = stage_w(wo.rearrange("(kc ck) m -> ck kc m", ck=128), [128, 4, DIM],
                   "wo_s")

    bo_f = consts.tile([1, DIM], FP32)
    nc.sync.dma_start(out=bo_f[:], in_=bo[None, :])
    bb_ps = ps.tile([128, 512], FP32, tag="ps")
    nc.tensor.matmul(bb_ps[:, 0:DIM], ones[0:1, :], bo_f[:], start=True, stop=True)
    bo_bc = consts.tile([128, DIM], FP32)
    nc.scalar.copy(bo_bc[:], bb_ps[:, 0:DIM])

    # persistent rotated buffers: kT2 (zero quadrants), v_raw (ones column),
    # expSz (zero rows outside each window's j-block, for K=113 av matmuls)
    kT2_bufs, v2_bufs, ez_bufs = [], [], []
    for i in range(NROT):
        t = consts.tile([128, 2, 4, 128], BF16, tag=f"kT2_{i}")
        nc.gpsimd.memset(t[:], 0.0)
        kT2_bufs.append(t)
        v = consts.tile([128, H, D + 1], BF16, tag=f"v2_{i}")
        nc.gpsimd.memset(v[:, :, D:D + 1], 1.0)
        v2_bufs.append(v)
        ez = []
        for hp in range(2):
            e = consts.tile([128, 2, 4, PP], BF16, tag=f"ez_{i}_{hp}")
            nc.gpsimd.memset(e[:], 0.0)
            ez.append(e)
        ez_bufs.append(ez)

    # ---- software-pipelined main loop (v5: per-engine readiness order) ----
    # Group g's stages: P at iter g-1, A at g, B at g+1, C1 at g+2,
    # C2 at g+3, D at g+4. Within an iteration, ops are emitted so that
    # every engine's in-order queue meets its dependencies without stalling:
    # PE runs [transp(i), st(i-1), av(i-2), otT(i-3), fin(i-4), qk+v(i)],
    # ACT runs [fT(i), exp(i-1), outT(i-3), kT2(i)],
    # DVE runs [recd+norm(i-2), fin-stt(i-4), qT2(i), v2(i)].
    n_groups = n_imgs * NW * (NW // 2)

    def coords(g):
        img, rem = divmod(g, NW * (NW // 2))
        wx, u = divmod(rem, NW // 2)
        return img, wx, u

    s = {}  # cross-stage state, keyed (group, name)

    def live(g):
        return 0 <= g < n_groups

    for i in range(n_groups + 5):
        if i == 0:
            em_dma_in(nc, sb, s, fm, coords(0), 0)
        if live(i + 1):
            em_dma_in(nc, sb, s, fm, coords(i + 1), i + 1)
        if live(i - 1):
            em_copies_tail(nc, sb, s, kT2_bufs[(i - 1) % NROT],
                           v2_bufs[(i - 1) % NROT], i - 1)
        if live(i):
            em_pe_transp(nc, sb, ps, s, identb, i)
            em_act_ft(nc, sb, s, i)
        if live(i - 2):
            em_pe_av(nc, ps, s, i - 2)
            em_dve_norm(nc, sb, s, i - 2)
        if live(i - 3):
            em_pe_ot(nc, ps, s, identb, i - 3)
            em_act_outT(nc, sb, s, i - 3)
        if live(i - 4):
            em_pe_fin(nc, ps, s, wo_s, i - 4)
            em_dve_fin(nc, sb, s, bo_bc, i - 4)
        if live(i - 1):
            em_pe_st(nc, ps, s, i - 1)
            em_act_exp(nc, sb, s, ez_bufs[(i - 1) % NROT], i - 1)
        if live(i):
            em_pe_qkv(nc, ps, s, wq_s, wk_s, wv_s, i)
        if live(i - 4):
            em_dma_out(nc, s, out, coords(i - 4), i - 4)


def em_dma_in(nc, sb, s, fm, c, g):
    img, wx, u = c
    f_raw = sb.tile([128, DIM], BF16, tag="f_raw")
    for w, eng in ((0, nc.gpsimd), (1, nc.sync)):
        c0 = P * (2 * u + w)
        eng.dma_start(out=f_raw[64 * w:64 * w + PP, :],
                      in_=fm[img, P * wx:P * wx + P, c0:c0 + P, :])
    s[(g, "f_raw")] = f_raw


def em_pe_transp(nc, sb, ps, s, identb, g):
    f_raw = s.pop((g, "f_raw"))
    fT_bank = ps.tile([128, 1024], BF16, tag="ps")
    fT_ps = fT_bank[:, 0:256].rearrange("p (kc t) -> p kc t", kc=2)
    for kc in range(2):
        nc.tensor.transpose(fT_ps[:, kc, 0:113],
                            f_raw[0:113, 128 * kc:128 * kc + 128],
                            identb[0:113, 0:113])
    s[(g, "fT_ps")] = fT_ps


def em_pe_st(nc, ps, s, g):
    qT2 = s.pop((g, "qT2"))
    kT2 = s.pop((g, "kT2"))
    st_bank = ps.tile([128, 512], FP32, tag="ps")
    st_ps = st_bank[:, 0:392].rearrange("p (w ch t) -> p w ch t", w=2, ch=4)
    for w in range(2):
        for ch in range(4):
            nc.tensor.matmul(st_ps[:, w, ch, :], kT2[:, w, ch, :],
                             qT2[:, ch, w, :], start=True, stop=True)
    s[(g, "st_ps")] = st_ps


def em_pe_av(nc, ps, s, g):
    ez = s.pop((g, "expSz"))
    v2 = s.pop((g, "v2"))
    # bank per hp; window w at col-block 64w. Rows of the bank then match
    # out_tok's token rows (p = 64w + i) identically, so normalization is
    # ONE stt per hp over [0:113] instead of four [49,...] ops.
    av_banks = []
    for hp in range(2):
        avb = ps.tile([128, 512], FP32, tag="ps")
        av = avb[:, 0:260].rearrange("p (ch e) -> p ch e", ch=4)
        av_banks.append(av)
        for w in range(2):
            for ch in range(4):
                h = 2 * ch + hp
                nc.tensor.matmul(
                    av[64 * w:64 * w + PP, ch, :],
                    ez[hp][0:113, w, ch, :],
                    v2[0:113, h, :],
                    tile_position=(0, 64 * w), start=True, stop=True)
    s[(g, "av")] = av_banks


def em_pe_ot(nc, ps, s, identb, g):
    out_tok = s.pop((g, "out_tok"))
    ot_flat = out_tok[:].rearrange("p ch hp d -> p (ch hp d)")
    ot_bank = ps.tile([128, 1024], BF16, tag="ps")
    ot_ps = ot_bank[:, 0:512].rearrange("p (nk t) -> p nk t", nk=4)
    for nk in range(4):
        nc.tensor.transpose(ot_ps[:, nk, 0:113],
                            ot_flat[0:113, 128 * nk:128 * nk + 128],
                            identb[0:113, 0:113])
    s[(g, "ot_ps")] = ot_ps


def em_pe_fin(nc, ps, s, wo_s, g):
    outT = s.pop((g, "outT"))
    fin_bank = ps.tile([128, 512], FP32, tag="ps")
    fin_ps = fin_bank[:, 0:DIM]
    for nk in range(4):
        nc.tensor.matmul(fin_ps[:], outT[:, nk, :], wo_s[:, nk, :],
                         start=(nk == 0), stop=(nk == 3))
    s[(g, "fin_ps")] = fin_ps


def em_act_ft(nc, sb, s, g):
    fT_ps = s.pop((g, "fT_ps"))
    fT = sb.tile([128, 2, 128], BF16, tag="fT")
    nc.scalar.copy(fT[:, 0, :], fT_ps[:, 0, :])
    nc.vector.tensor_copy(fT[:, 1, :], fT_ps[:, 1, :])
    s[(g, "fT")] = fT


def em_act_exp(nc, sb, s, ez, g):
    st_ps = s.pop((g, "st_ps"))
    for hp in range(2):
        for w in range(2):
            nc.scalar.activation(
                ez[hp][64 * w:64 * w + PP, w, :, :],
                st_ps[64 * hp:64 * hp + PP, w, :, :],
                mybir.ActivationFunctionType.Exp, scale=SCALE)
    s[(g, "expSz")] = ez


def em_act_outT(nc, sb, s, g):
    ot_ps = s.pop((g, "ot_ps"))
    outT = sb.tile([128, 4, 128], BF16, tag="outT")
    nc.vector.tensor_copy(outT[:], ot_ps[:])
    s[(g, "outT")] = outT


def em_dve_norm(nc, sb, s, g):
    av_banks = s.pop((g, "av"))
    out_tok = sb.tile([128, 4, 2, D], BF16, tag="out_tok")  # free = (ch, hp, d)
    for hp in range(2):
        av = av_banks[hp]
        recd = sb.tile([128, 4, 1], FP32, tag=f"recd{hp}")
        nc.vector.reciprocal(recd[0:113, :, :], av[0:113, :, D:D + 1])
        nc.vector.scalar_tensor_tensor(
            out=out_tok[0:113, :, hp, :],
            in0=av[0:113, :, 0:D],
            scalar=1.0,
            in1=recd[0:113, :, 0:1].broadcast_to((113, 4, D)),
            op0=mybir.AluOpType.mult, op1=mybir.AluOpType.mult)
    s[(g, "out_tok")] = out_tok


def em_dve_fin(nc, sb, s, bo_bc, g):
    fin_ps = s.pop((g, "fin_ps"))
    fin = sb.tile([128, DIM], FP32, tag="fin")
    nc.vector.scalar_tensor_tensor(out=fin[:], in0=fin_ps[:], scalar=1.0,
                                   in1=bo_bc[:], op0=mybir.AluOpType.mult,
                                   op1=mybir.AluOpType.add)
    s[(g, "fin")] = fin


def em_pe_qkv(nc, ps, s, wq_s, wk_s, wv_s, g):
    fT = s.pop((g, "fT"))
    fT_c = fT[:].rearrange("p kc (w ts) -> p kc w ts", w=2)[:, :, :, 0:PP]
    q_bank = ps.tile([128, 512], FP32, tag="ps")
    qT_ps = q_bank[:, 0:392].rearrange("p (nk w t) -> p nk w t", nk=4, w=2)
    k_bank = ps.tile([128, 512], FP32, tag="ps")
    kT_ps = k_bank[:, 0:392].rearrange("p (nk w t) -> p nk w t", nk=4, w=2)
    for nk in range(4):
        for kc in range(2):
            nc.tensor.matmul(qT_ps[:, nk, :, :],
                             wq_s[:, kc, 128 * nk:128 * nk + 128],
                             fT_c[:, kc, :, :], start=(kc == 0), stop=(kc == 1))
            nc.tensor.matmul(kT_ps[:, nk, :, :],
                             wk_s[:, kc, 128 * nk:128 * nk + 128],
                             fT_c[:, kc, :, :], start=(kc == 0), stop=(kc == 1))
    v_bank = ps.tile([128, 512], FP32, tag="ps")
    for kc in range(2):
        nc.tensor.matmul(v_bank[:], fT[:, kc, :], wv_s[:, kc, :],
                         start=(kc == 0), stop=(kc == 1))
    s[(g, "qT_ps")] = qT_ps
    s[(g, "kT_ps")] = kT_ps
    s[(g, "v_ps")] = v_bank


def em_copies_tail(nc, sb, s, kT2, v2, g):
    qT_ps = s.pop((g, "qT_ps"))
    kT_ps = s.pop((g, "kT_ps"))
    v_ps = s.pop((g, "v_ps"))

    qT2 = sb.tile([128, 4, 2, PP], BF16, tag="qT2")
    nc.vector.tensor_copy(qT2[:], qT_ps[:])
    for hp in range(2):
        nc.scalar.copy(
            kT2[64 * hp:64 * hp + 64, :, :, 64 * hp:64 * hp + PP],
            kT_ps[64 * hp:64 * hp + 64, :, :, :].rearrange(
                "p nk w ts -> p w nk ts"))
    nc.vector.tensor_copy(v2[0:113, :, 0:D],
                          v_ps[0:113, :].rearrange("p (h d) -> p h d", h=H))
    s[(g, "qT2")] = qT2
    s[(g, "kT2")] = kT2
    s[(g, "v2")] = v2


def em_dma_out(nc, s, out, c, g):
    img, wx, u = c
    fin = s.pop((g, "fin"))
    for w, eng in ((0, nc.sync), (1, nc.gpsimd)):
        c0 = P * (2 * u + w)
        eng.dma_start(out=out[img, P * wx:P * wx + P, c0:c0 + P, :],
                      in_=fin[64 * w:64 * w + PP, :])


_CACHED = {}


def _get_nc():
    if "nc" not in _CACHED:
        _CACHED["nc"] = build_bass()
    return _CACHED["nc"]


def kernel(fmap, Wq, Wkv, Wo, bo, _trace=False, _trace_kwargs=None):
    import ml_dtypes
    fmap = np.ascontiguousarray(fmap).astype(ml_dtypes.bfloat16)
    nc = _get_nc()
    in_maps = []
    for c in range(NCORES):
        in_maps.append({
            "fmap": fmap[IMGS_PER_CORE * c:IMGS_PER_CORE * (c + 1)],
            "Wq": np.ascontiguousarray(Wq, dtype=np.float32),
            "Wkv": np.ascontiguousarray(Wkv, dtype=np.float32),
            "Wo": np.ascontiguousarray(Wo, dtype=np.float32),
            "bo": np.ascontiguousarray(bo, dtype=np.float32),
        })
    res = run_bass_kernel_spmd(nc, in_maps, core_ids=list(range(NCORES)),
                               trace=_trace, **(_trace_kwargs or {}))
    outs = [r["out"] for r in res.results]
    full = np.concatenate(outs, axis=0)
    if _trace:
        return full, res
    return full


# revision 29
# speedup vs baseline: 1.2082x; 1.2082x over previous
"""Local window attention (7x7 windows, 8 heads, d=64) Trainium2 Bass kernel.

Full inputs in, full outputs out. Data-parallel over batch across 8 cores
(4 images per core). Shapes hardcoded per spec:
  fmap (32, 56, 56, 256) f32, Wq (256,512), Wkv (256,1024), Wo (512,256), bo (256,)

Final design (2.88ms baseline -> 0.543ms, 5.3x). Key facts learned on HW:
engines execute their queues IN ORDER and the tile scheduler follows
emission order, so cross-group overlap must be software-pipelined
explicitly, with each engine's per-iteration queue sorted by dependency
readiness. fp32 matmuls are 4 cy/row vs bf16's 1. DMA triggers cost
~700-800ns on the issuing sequencer (spread across sync/gpsimd/scalar
queues). GPSIMD tensor ops are ~3x slower than DVE/ACT and cannot touch
PSUM. DVE reciprocal costs ~7.4ns per FREE element (keep free dims tiny).
Engine-op partition ranges must start 32-aligned; matmul operands must
start at partition 0 (high-half streaming bug).

Per group g = 2 adjacent-y windows, tokens 64-padded (p = 64w+7r+t); fmap
is cast to bf16 on the HOST (exec-time is NEFF time; halves input DMA).
Stages, emitted with skews P:+1 A:0 B:-1 C1:-2 C2:-3 D:-4 (6 groups in
flight, 9 PSUM bank allocs per iteration on the 8-bank rotation):

  P:  2 input DMAs ([49,256] <- [7,7,256]; gpsimd+sync queues)
  A:  2 bf16 PE transposes (N=113) -> fT (copy split ACT/DVE); q/k
      projections (16 mm, N=98 compact via strided rhs); qT2 one
      full-partition DVE copy; kT2 block-diag over 2 heads (2 ACT copies,
      zero quadrants persist in rotated buffers); v (2 mm) -> one
      full-partition ACT copy into v_raw (persistent ones column at 64)
  B:  ST 2-heads-per-matmul (8 mm, K=128 d-stack, N=49); exp: 4 ACT ops
      into a zero-padded arena expSz (w0 rows 0:49, w1 rows 64:113,
      zeros elsewhere kill junk in the K=113 av contraction)
  C1: av+denom (16 mm, K=113, N=65, lhsT=expSz, rhs=v_raw|1); tiny
      reciprocal of the denom column (free size 4!); normalize into
      out_tok via scalar_tensor_tensor with stride-0 broadcast_to of 1/den
  C2: 4 bf16 PE transposes (N=113) -> outT (DVE copy)
  D:  fin = outT.T @ Wo + bo (4 mm + DVE stt); 2 output DMAs (sync+gpsimd)

All matmuls bf16 (psum fp32); casts ride the mandatory psum->SBUF copies.
"""

from contextlib import ExitStack

import numpy as np

import concourse.bacc as bacc
import concourse.bass as bass
import concourse.tile as tile
from concourse import mybir
from concourse import bass_isa
from concourse.masks import make_identity
from concourse.bass_utils import run_bass_kernel_spmd

P = 7
PP = 49          # tokens per window
H = 8            # heads
D = 64           # head dim
DIM = 256        # channels
INNER = 512      # h*d
SCALE = D ** -0.5
IMGS_PER_CORE = 4
NCORES = 8
X = 56
NW = X // P      # 8 windows per axis
FP32 = mybir.dt.float32
BF16 = mybir.dt.bfloat16
NROT = 4         # manual rotation depth for persistent tiles


def build_bass(n_imgs=IMGS_PER_CORE):
    nc = bacc.Bacc("TRN2", target_bir_lowering=False, debug=False)

    fm = nc.dram_tensor("fmap", [n_imgs, X, X, DIM], BF16, kind="ExternalInput").ap()
    wq = nc.dram_tensor("Wq", [DIM, INNER], FP32, kind="ExternalInput").ap()
    wkv = nc.dram_tensor("Wkv", [DIM, 2 * INNER], FP32, kind="ExternalInput").ap()
    wo = nc.dram_tensor("Wo", [INNER, DIM], FP32, kind="ExternalInput").ap()
    bo = nc.dram_tensor("bo", [DIM], FP32, kind="ExternalInput").ap()
    out = nc.dram_tensor("out", [n_imgs, X, X, DIM], FP32, kind="ExternalOutput").ap()

    with tile.TileContext(nc) as tc:
        with ExitStack() as ctx:
            build_kernel(ctx, tc, out, fm, wq, wkv, wo, bo, n_imgs)
    nc.compile()
    return nc


def build_kernel(ctx, tc, out, fm, wq, wkv, wo, bo, n_imgs=IMGS_PER_CORE):
    nc = tc.nc
    consts = ctx.enter_context(tc.tile_pool(name="consts", bufs=1))
    sb = ctx.enter_context(tc.tile_pool(name="sb", bufs=6))
    ps = ctx.enter_context(tc.tile_pool(name="ps", bufs=8, space="PSUM"))

    # ---- constants ----
    ident = consts.tile([128, 128], FP32)
    make_identity(nc, ident[:])

    ones = consts.tile([1, 128], FP32)
    nc.gpsimd.memset(ones[:], 1.0)
    identb = consts.tile([128, 128], BF16)
    nc.vector.tensor_copy(identb[:], ident[:])

    def stage_w(dram_ap, shape, name):
        st = sb.tile(shape, FP32, tag="stage")
        nc.sync.dma_start(out=st[:], in_=dram_ap)
        bt = consts.tile(shape, BF16, tag=name)
        nc.vector.tensor_copy(bt[:], st[:])
        return bt

    wq_s = stage_w(wq.rearrange("(kc ck) n -> ck kc n", ck=128), [128, 2, INNER],
                   "wq_s")
    wk_s = stage_w(wkv[:, 0:INNER].rearrange("(kc ck) n -> ck kc n", ck=128),
                   [128, 2, INNER], "wk_s")
    wv_s = stage_w(wkv[:, INNER:2 * INNER].rearrange("(kc ck) n -> ck kc n", ck=128),
                   [128, 2, INNER], "wv_s")
    wo_s = stage_w(wo.rearrange("(kc ck) m -> ck kc m", ck=128), [128, 4, DIM],
                   "wo_s")

    bo_f = consts.tile([1, DIM], FP32)
    nc.sync.dma_start(out=bo_f[:], in_=bo[None, :])
    bb_ps = ps.tile([128, 512], FP32, tag="ps")
    nc.tensor.matmul(bb_ps[:, 0:DIM], ones[0:1, :], bo_f[:], start=True, stop=True)
    bo_bc = consts.tile([128, DIM], FP32)
    nc.scalar.copy(bo_bc[:], bb_ps[:, 0:DIM])

    # persistent rotated buffers: kT2 (zero quadrants), v_raw (ones column),
    # expSz (zero rows outside each window's j-block, for K=113 av matmuls)
    kT2_bufs, v2_bufs, ez_bufs = [], [], []
    for i in range(NROT):
        t = consts.tile([128, 2, 4, 128], BF16, tag=f"kT2_{i}")
        nc.gpsimd.memset(t[:], 0.0)
        kT2_bufs.append(t)
        v = consts.tile([128, H, D + 1], BF16, tag=f"v2_{i}")
        nc.gpsimd.memset(v[:, :, D:D + 1], 1.0)
        v2_bufs.append(v)
        ez = []
        for hp in range(2):
            e = consts.tile([128, 2, 4, PP], BF16, tag=f"ez_{i}_{hp}")
            nc.gpsimd.memset(e[:], 0.0)
            ez.append(e)
        ez_bufs.append(ez)

    # ---- software-pipelined main loop (v5: per-engine readiness order) ----
    # Group g's stages: P at iter g-1, A at g, B at g+1, C1 at g+2,
    # C2 at g+3, D at g+4. Within an iteration, ops are emitted so that
    # every engine's in-order queue meets its dependencies without stalling:
    # PE runs [transp(i), st(i-1), av(i-2), otT(i-3), fin(i-4), qk+v(i)],
    # ACT runs [fT(i), exp(i-1), outT(i-3), kT2(i)],
    # DVE runs [recd+norm(i-2), fin-stt(i-4), qT2(i), v2(i)].
    n_groups = n_imgs * NW * (NW // 2)

    def coords(g):
        img, rem = divmod(g, NW * (NW // 2))
        wx, u = divmod(rem, NW // 2)
        return img, wx, u

    s = {}  # cross-stage state, keyed (group, name)

    def live(g):
        return 0 <= g < n_groups

    for i in range(n_groups + 5):
        if i == 0:
            em_dma_in(nc, sb, s, fm, coords(0), 0)
        if live(i + 1):
            em_dma_in(nc, sb, s, fm, coords(i + 1), i + 1)
        if live(i - 1):
            em_copies_tail(nc, sb, s, kT2_bufs[(i - 1) % NROT],
                           v2_bufs[(i - 1) % NROT], i - 1)
        if live(i):
            em_pe_transp(nc, sb, ps, s, identb, i)
            em_act_ft(nc, sb, s, i)
        if live(i - 2):
            em_pe_av(nc, ps, s, i - 2)
            em_dve_norm(nc, sb, s, i - 2)
        if live(i - 3):
            em_pe_ot(nc, ps, s, identb, i - 3)
            em_act_outT(nc, sb, s, i - 3)
        if live(i - 4):
            em_pe_fin(nc, ps, s, wo_s, i - 4)
            em_dve_fin(nc, sb, s, bo_bc, i - 4)
        if live(i - 1):
            em_pe_st(nc, ps, s, i - 1)
            em_act_exp(nc, sb, s, ez_bufs[(i - 1) % NROT], i - 1)
        if live(i):
            em_pe_qkv(nc, ps, s, wq_s, wk_s, wv_s, i)
        if live(i - 4):
            em_dma_out(nc, s, out, coords(i - 4), i - 4)


def em_dma_in(nc, sb, s, fm, c, g):
    img, wx, u = c
    f_raw = sb.tile([128, DIM], BF16, tag="f_raw")
    for w, eng in ((0, nc.gpsimd), (1, nc.sync)):
        c0 = P * (2 * u + w)
        eng.dma_start(out=f_raw[64 * w:64 * w + PP, :],
                      in_=fm[img, P * wx:P * wx + P, c0:c0 + P, :])
    s[(g, "f_raw")] = f_raw


def em_pe_transp(nc, sb, ps, s, identb, g):
    f_raw = s.pop((g, "f_raw"))
    fT_bank = ps.tile([128, 1024], BF16, tag="ps")
    fT_ps = fT_bank[:, 0:256].rearrange("p (kc t) -> p kc t", kc=2)
    for kc in range(2):
        nc.tensor.transpose(fT_ps[:, kc, 0:113],
                            f_raw[0:113, 128 * kc:128 * kc + 128],
                            identb[0:113, 0:113])
    s[(g, "fT_ps")] = fT_ps


def em_pe_st(nc, ps, s, g):
    qT2 = s.pop((g, "qT2"))
    kT2 = s.pop((g, "kT2"))
    st_bank = ps.tile([128, 512], FP32, tag="ps")
    st_ps = st_bank[:, 0:392].rearrange("p (w ch t) -> p w ch t", w=2, ch=4)
    for w in range(2):
        for ch in range(4):
            nc.tensor.matmul(st_ps[:, w, ch, :], kT2[:, w, ch, :],
                             qT2[:, ch, w, :], start=True, stop=True)
    s[(g, "st_ps")] = st_ps


def em_pe_av(nc, ps, s, g):
    ez = s.pop((g, "expSz"))
    v2 = s.pop((g, "v2"))
    # bank per hp; window w at col-block 64w. Rows of the bank then match
    # out_tok's token rows (p = 64w + i) identically, so normalization is
    # ONE stt per hp over [0:113] instead of four [49,...] ops.
    av_banks = []
    for hp in range(2):
        avb = ps.tile([128, 512], FP32, tag="ps")
        av = avb[:, 0:260].rearrange("p (ch e) -> p ch e", ch=4)
        av_banks.append(av)
        for w in range(2):
            for ch in range(4):
                h = 2 * ch + hp
                nc.tensor.matmul(
                    av[64 * w:64 * w + PP, ch, :],
                    ez[hp][0:113, w, ch, :],
                    v2[0:113, h, :],
                    tile_position=(0, 64 * w), start=True, stop=True)
    s[(g, "av")] = av_banks


def em_pe_ot(nc, ps, s, identb, g):
    out_tok = s.pop((g, "out_tok"))
    ot_flat = out_tok[:].rearrange("p ch hp d -> p (ch hp d)")
    ot_bank = ps.tile([128, 1024], BF16, tag="ps")
    ot_ps = ot_bank[:, 0:512].rearrange("p (nk t) -> p nk t", nk=4)
    for nk in range(4):
        nc.tensor.transpose(ot_ps[:, nk, 0:113],
                            ot_flat[0:113, 128 * nk:128 * nk + 128],
                            identb[0:113, 0:113])
    s[(g, "ot_ps")] = ot_ps


def em_pe_fin(nc, ps, s, wo_s, g):
    outT = s.pop((g, "outT"))
    fin_bank = ps.tile([128, 512], FP32, tag="ps")
    fin_ps = fin_bank[:, 0:DIM]
    for nk in range(4):
        nc.tensor.matmul(fin_ps[:], outT[:, nk, :], wo_s[:, nk, :],
                         start=(nk == 0), stop=(nk == 3))
    s[(g, "fin_ps")] = fin_ps


def em_act_ft(nc, sb, s, g):
    fT_ps = s.pop((g, "fT_ps"))
    fT = sb.tile([128, 2, 128], BF16, tag="fT")
    nc.scalar.copy(fT[:, 0, :], fT_ps[:, 0, :])
    nc.vector.tensor_copy(fT[:, 1, :], fT_ps[:, 1, :])
    s[(g, "fT")] = fT


def em_act_exp(nc, sb, s, ez, g):
    st_ps = s.pop((g, "st_ps"))
    for hp in range(2):
        for w in range(2):
            nc.scalar.activation(
                ez[hp][64 * w:64 * w + PP, w, :, :],
                st_ps[64 * hp:64 * hp + PP, w, :, :],
                mybir.ActivationFunctionType.Exp, scale=SCALE)
    s[(g, "expSz")] = ez


def em_act_outT(nc, sb, s, g):
    ot_ps = s.pop((g, "ot_ps"))
    outT = sb.tile([128, 4, 128], BF16, tag="outT")
    nc.vector.tensor_copy(outT[:], ot_ps[:])
    s[(g, "outT")] = outT


def em_dve_norm(nc, sb, s, g):
    av_banks = s.pop((g, "av"))
    out_tok = sb.tile([128, 4, 2, D], BF16, tag="out_tok")  # free = (ch, hp, d)
    for hp in range(2):
        av = av_banks[hp]
        recd = sb.tile([128, 4, 1], FP32, tag=f"recd{hp}")
        nc.vector.reciprocal(recd[0:113, :, :], av[0:113, :, D:D + 1])
        nc.vector.scalar_tensor_tensor(
            out=out_tok[0:113, :, hp, :],
            in0=av[0:113, :, 0:D],
            scalar=1.0,
            in1=recd[0:113, :, 0:1].broadcast_to((113, 4, D)),
            op0=mybir.AluOpType.mult, op1=mybir.AluOpType.mult)
    s[(g, "out_tok")] = out_tok


def em_dve_fin(nc, sb, s, bo_bc, g):
    fin_ps = s.pop((g, "fin_ps"))
    fin = sb.tile([128, DIM], FP32, tag="fin")
    nc.vector.scalar_tensor_tensor(out=fin[:], in0=fin_ps[:], scalar=1.0,
                                   in1=bo_bc[:], op0=mybir.AluOpType.mult,
                                   op1=mybir.AluOpType.add)
    s[(g, "fin")] = fin


def em_pe_qkv(nc, ps, s, wq_s, wk_s, wv_s, g):
    fT = s.pop((g, "fT"))
    fT_c = fT[:].rearrange("p kc (w ts) -> p kc w ts", w=2)[:, :, :, 0:PP]
    q_bank = ps.tile([128, 512], FP32, tag="ps")
    qT_ps = q_bank[:, 0:392].rearrange("p (nk w t) -> p nk w t", nk=4, w=2)
    k_bank = ps.tile([128, 512], FP32, tag="ps")
    kT_ps = k_bank[:, 0:392].rearrange("p (nk w t) -> p nk w t", nk=4, w=2)
    for nk in range(4):
        for kc in range(2):
            nc.tensor.matmul(qT_ps[:, nk, :, :],
                             wq_s[:, kc, 128 * nk:128 * nk + 128],
                             fT_c[:, kc, :, :], start=(kc == 0), stop=(kc == 1))
            nc.tensor.matmul(kT_ps[:, nk, :, :],
                             wk_s[:, kc, 128 * nk:128 * nk + 128],
                             fT_c[:, kc, :, :], start=(kc == 0), stop=(kc == 1))
    v_bank = ps.tile([128, 512], FP32, tag="ps")
    for kc in range(2):
        nc.tensor.matmul(v_bank[:], fT[:, kc, :], wv_s[:, kc, :],
                         start=(kc == 0), stop=(kc == 1))
    s[(g, "qT_ps")] = qT_ps
    s[(g, "kT_ps")] = kT_ps
    s[(g, "v_ps")] = v_bank


def em_copies_tail(nc, sb, s, kT2, v2, g):
    qT_ps = s.pop((g, "qT_ps"))
    kT_ps = s.pop((g, "kT_ps"))
    v_ps = s.pop((g, "v_ps"))

    qT2 = sb.tile([128, 4, 2, PP], BF16, tag="qT2")
    nc.vector.tensor_copy(qT2[:], qT_ps[:])
    for hp in range(2):
        nc.scalar.copy(
            kT2[64 * hp:64 * hp + 64, :, :, 64 * hp:64 * hp + PP],
            kT_ps[64 * hp:64 * hp + 64, :, :, :].rearrange(
                "p nk w ts -> p w nk ts"))
    nc.scalar.copy(v2[0:113, :, 0:D],
                   v_ps[0:113, :].rearrange("p (h d) -> p h d", h=H))
    s[(g, "qT2")] = qT2
    s[(g, "kT2")] = kT2
    s[(g, "v2")] = v2


def em_dma_out(nc, s, out, c, g):
    img, wx, u = c
    fin = s.pop((g, "fin"))
    for w, eng in ((0, nc.sync), (1, nc.gpsimd)):
        c0 = P * (2 * u + w)
        eng.dma_start(out=out[img, P * wx:P * wx + P, c0:c0 + P, :],
                      in_=fin[64 * w:64 * w + PP, :])


_CACHED = {}


def _get_nc():
    if "nc" not in _CACHED:
        _CACHED["nc"] = build_bass()
    return _CACHED["nc"]


def kernel(fmap, Wq, Wkv, Wo, bo, _trace=False, _trace_kwargs=None):
    import ml_dtypes
    fmap = np.ascontiguousarray(fmap).astype(ml_dtypes.bfloat16)
    nc = _get_nc()
    in_maps = []
    for c in range(NCORES):
        in_maps.append({
            "fmap": fmap[IMGS_PER_CORE * c:IMGS_PER_CORE * (c + 1)],
            "Wq": np.ascontiguousarray(Wq, dtype=np.float32),
            "Wkv": np.ascontiguousarray(Wkv, dtype=np.float32),
            "Wo": np.ascontiguousarray(Wo, dtype=np.float32),
            "bo": np.ascontiguousarray(bo, dtype=np.float32),
        })
    res = run_bass_kernel_spmd(nc, in_maps, core_ids=list(range(NCORES)),
                               trace=_trace, **(_trace_kwargs or {}))
    outs = [r["out"] for r in res.results]
    full = np.concatenate(outs, axis=0)
    if _trace:
        return full, res
    return full


# revision 31
# speedup vs baseline: 1.2279x; 1.0163x over previous
"""Local window attention (7x7 windows, 8 heads, d=64) Trainium2 Bass kernel.

Full inputs in, full outputs out. Data-parallel over batch across 8 cores
(4 images per core). Shapes hardcoded per spec:
  fmap (32, 56, 56, 256) f32, Wq (256,512), Wkv (256,1024), Wo (512,256), bo (256,)

Final design (2.88ms baseline -> 0.543ms, 5.3x). Key facts learned on HW:
engines execute their queues IN ORDER and the tile scheduler follows
emission order, so cross-group overlap must be software-pipelined
explicitly, with each engine's per-iteration queue sorted by dependency
readiness. fp32 matmuls are 4 cy/row vs bf16's 1. DMA triggers cost
~700-800ns on the issuing sequencer (spread across sync/gpsimd/scalar
queues). GPSIMD tensor ops are ~3x slower than DVE/ACT and cannot touch
PSUM. DVE reciprocal costs ~7.4ns per FREE element (keep free dims tiny).
Engine-op partition ranges must start 32-aligned; matmul operands must
start at partition 0 (high-half streaming bug).

Per group g = 2 adjacent-y windows, tokens 64-padded (p = 64w+7r+t); fmap
is cast to bf16 on the HOST (exec-time is NEFF time; halves input DMA).
Stages, emitted with skews P:+1 A:0 B:-1 C1:-2 C2:-3 D:-4 (6 groups in
flight, 9 PSUM bank allocs per iteration on the 8-bank rotation):

  P:  2 input DMAs ([49,256] <- [7,7,256]; gpsimd+sync queues)
  A:  2 bf16 PE transposes (N=113) -> fT (copy split ACT/DVE); q/k
      projections (16 mm, N=98 compact via strided rhs); qT2 one
      full-partition DVE copy; kT2 block-diag over 2 heads (2 ACT copies,
      zero quadrants persist in rotated buffers); v (2 mm) -> one
      full-partition ACT copy into v_raw (persistent ones column at 64)
  B:  ST 2-heads-per-matmul (8 mm, K=128 d-stack, N=49); exp: 4 ACT ops
      into a zero-padded arena expSz (w0 rows 0:49, w1 rows 64:113,
      zeros elsewhere kill junk in the K=113 av contraction)
  C1: av+denom (16 mm, K=113, N=65, lhsT=expSz, rhs=v_raw|1); tiny
      reciprocal of the denom column (free size 4!); normalize into
      out_tok via scalar_tensor_tensor with stride-0 broadcast_to of 1/den
  C2: 4 bf16 PE transposes (N=113) -> outT (DVE copy)
  D:  fin = outT.T @ Wo + bo (4 mm + DVE stt); 2 output DMAs (sync+gpsimd)

All matmuls bf16 (psum fp32); casts ride the mandatory psum->SBUF copies.
"""

from contextlib import ExitStack

import numpy as np

import concourse.bacc as bacc
import concourse.bass as bass
import concourse.tile as tile
from concourse import mybir
from concourse import bass_isa
from concourse.masks import make_identity
from concourse.bass_utils import run_bass_kernel_spmd

P = 7
PP = 49          # tokens per window
H = 8            # heads
D = 64           # head dim
DIM = 256        # channels
INNER = 512      # h*d
SCALE = D ** -0.5
IMGS_PER_CORE = 4
NCORES = 8
X = 56
NW = X // P      # 8 windows per axis
FP32 = mybir.dt.float32
BF16 = mybir.dt.bfloat16
NROT = 4         # manual rotation depth for persistent tiles


def build_bass(n_imgs=IMGS_PER_CORE):
    nc = bacc.Bacc("TRN2", target_bir_lowering=False, debug=False)

    fm = nc.dram_tensor("fmap", [n_imgs, X, X, DIM], BF16, kind="ExternalInput").ap()
    wq = nc.dram_tensor("Wq", [DIM, INNER], FP32, kind="ExternalInput").ap()
    wkv = nc.dram_tensor("Wkv", [DIM, 2 * INNER], FP32, kind="ExternalInput").ap()
    wo = nc.dram_tensor("Wo", [INNER, DIM], FP32, kind="ExternalInput").ap()
    bo = nc.dram_tensor("bo", [DIM], FP32, kind="ExternalInput").ap()
    out = nc.dram_tensor("out", [n_imgs, X, X, DIM], FP32, kind="ExternalOutput").ap()

    with tile.TileContext(nc) as tc:
        with ExitStack() as ctx:
            build_kernel(ctx, tc, out, fm, wq, wkv, wo, bo, n_imgs)
    nc.compile()
    return nc


def build_kernel(ctx, tc, out, fm, wq, wkv, wo, bo, n_imgs=IMGS_PER_CORE):
    nc = tc.nc
    consts = ctx.enter_context(tc.tile_pool(name="consts", bufs=1))
    sb = ctx.enter_context(tc.tile_pool(name="sb", bufs=6))
    ps = ctx.enter_context(tc.tile_pool(name="ps", bufs=8, space="PSUM"))

    # ---- constants ----
    ident = consts.tile([128, 128], FP32)
    make_identity(nc, ident[:])

    ones = consts.tile([1, 128], FP32)
    nc.gpsimd.memset(ones[:], 1.0)
    identb = consts.tile([128, 128], BF16)
    nc.vector.tensor_copy(identb[:], ident[:])

    def stage_w(dram_ap, shape, name):
        st = sb.tile(shape, FP32, tag="stage")
        nc.sync.dma_start(out=st[:], in_=dram_ap)
        bt = consts.tile(shape, BF16, tag=name)
        nc.vector.tensor_copy(bt[:], st[:])
        return bt

    wq_s = stage_w(wq.rearrange("(kc ck) n -> ck kc n", ck=128), [128, 2, INNER],
                   "wq_s")
    wk_s = stage_w(wkv[:, 0:INNER].rearrange("(kc ck) n -> ck kc n", ck=128),
                   [128, 2, INNER], "wk_s")
    wv_s = stage_w(wkv[:, INNER:2 * INNER].rearrange("(kc ck) n -> ck kc n", ck=128),
                   [128, 2, INNER], "wv_s")
    wo_s = stage_w(wo.rearrange("(kc ck) m -> ck kc m", ck=128), [128, 4, DIM],
                   "wo_s")

    bo_f = consts.tile([1, DIM], FP32)
    nc.sync.dma_start(out=bo_f[:], in_=bo[None, :])
    bb_ps = ps.tile([128, 512], FP32, tag="ps")
    nc.tensor.matmul(bb_ps[:, 0:DIM], ones[0:1, :], bo_f[:], start=True, stop=True)
    bo_bc = consts.tile([128, DIM], FP32)
    nc.scalar.copy(bo_bc[:], bb_ps[:, 0:DIM])

    # persistent rotated buffers: kT2 (zero quadrants), v_raw (ones column),
    # expSz (zero rows outside each window's j-block, for K=113 av matmuls)
    kT2_bufs, v2_bufs, ez_bufs = [], [], []
    for i in range(NROT):
        t = consts.tile([128, 2, 4, 128], BF16, tag=f"kT2_{i}")
        nc.gpsimd.memset(t[:], 0.0)
        kT2_bufs.append(t)
        v = consts.tile([128, H, D + 1], BF16, tag=f"v2_{i}")
        nc.gpsimd.memset(v[:, :, D:D + 1], 1.0)
        v2_bufs.append(v)
        ez = []
        for hp in range(2):
            e = consts.tile([128, 2, 4, PP], BF16, tag=f"ez_{i}_{hp}")
            nc.gpsimd.memset(e[:], 0.0)
            ez.append(e)
        ez_bufs.append(ez)

    # ---- software-pipelined main loop (v5: per-engine readiness order) ----
    # Group g's stages: P at iter g-1, A at g, B at g+1, C1 at g+2,
    # C2 at g+3, D at g+4. Within an iteration, ops are emitted so that
    # every engine's in-order queue meets its dependencies without stalling:
    # PE runs [transp(i), st(i-1), av(i-2), otT(i-3), fin(i-4), qk+v(i)],
    # ACT runs [fT(i), exp(i-1), outT(i-3), kT2(i)],
    # DVE runs [recd+norm(i-2), fin-stt(i-4), qT2(i), v2(i)].
    n_groups = n_imgs * NW * (NW // 2)

    def coords(g):
        img, rem = divmod(g, NW * (NW // 2))
        wx, u = divmod(rem, NW // 2)
        return img, wx, u

    s = {}  # cross-stage state, keyed (group, name)

    def live(g):
        return 0 <= g < n_groups

    for i in range(n_groups + 5):
        if i == 0:
            em_dma_in(nc, sb, s, fm, coords(0), 0)
        if live(i + 1):
            em_dma_in(nc, sb, s, fm, coords(i + 1), i + 1)
        if live(i - 1):
            em_copies_tail(nc, sb, s, kT2_bufs[(i - 1) % NROT],
                           v2_bufs[(i - 1) % NROT], i - 1)
        if live(i):
            em_pe_transp(nc, sb, ps, s, identb, i)
            em_act_ft(nc, sb, s, i)
        if live(i - 2):
            em_pe_av(nc, ps, s, i - 2)
            em_dve_norm(nc, sb, s, i - 2)
        if live(i - 3):
            em_pe_ot(nc, ps, s, identb, i - 3)
            em_act_outT(nc, sb, s, i - 3)
        if live(i - 4):
            em_pe_fin(nc, ps, s, wo_s, i - 4)
            em_dve_fin(nc, sb, s, bo_bc, i - 4)
        if live(i - 1):
            em_pe_st(nc, ps, s, i - 1)
            em_act_exp(nc, sb, s, ez_bufs[(i - 1) % NROT], i - 1)
        if live(i):
            em_pe_qkv(nc, ps, s, wq_s, wk_s, wv_s, i)
        if live(i - 4):
            em_dma_out(nc, s, out, coords(i - 4), i - 4)


def em_dma_in(nc, sb, s, fm, c, g):
    img, wx, u = c
    f_raw = sb.tile([128, DIM], BF16, tag="f_raw")
    for w, eng in ((0, nc.gpsimd), (1, nc.sync)):
        c0 = P * (2 * u + w)
        eng.dma_start(out=f_raw[64 * w:64 * w + PP, :],
                      in_=fm[img, P * wx:P * wx + P, c0:c0 + P, :])
    s[(g, "f_raw")] = f_raw


def em_pe_transp(nc, sb, ps, s, identb, g):
    f_raw = s.pop((g, "f_raw"))
    fT_bank = ps.tile([128, 1024], BF16, tag="ps")
    fT_ps = fT_bank[:, 0:256].rearrange("p (kc t) -> p kc t", kc=2)
    for kc in range(2):
        nc.tensor.transpose(fT_ps[:, kc, 0:113],
                            f_raw[0:113, 128 * kc:128 * kc + 128],
                            identb[0:113, 0:113])
    s[(g, "fT_ps")] = fT_ps


def em_pe_st(nc, ps, s, g):
    qT2 = s.pop((g, "qT2"))
    kT2 = s.pop((g, "kT2"))
    st_bank = ps.tile([128, 512], FP32, tag="ps")
    st_ps = st_bank[:, 0:392].rearrange("p (w ch t) -> p w ch t", w=2, ch=4)
    for w in range(2):
        for ch in range(4):
            nc.tensor.matmul(st_ps[:, w, ch, :], kT2[:, w, ch, :],
                             qT2[:, ch, w, :], start=True, stop=True)
    s[(g, "st_ps")] = st_ps


def em_pe_av(nc, ps, s, g):
    ez = s.pop((g, "expSz"))
    v2 = s.pop((g, "v2"))
    # bank per hp; window w at col-block 64w. Rows of the bank then match
    # out_tok's token rows (p = 64w + i) identically, so normalization is
    # ONE stt per hp over [0:113] instead of four [49,...] ops.
    av_banks = []
    for hp in range(2):
        avb = ps.tile([128, 512], FP32, tag="ps")
        av = avb[:, 0:260].rearrange("p (ch e) -> p ch e", ch=4)
        av_banks.append(av)
        for w in range(2):
            for ch in range(4):
                h = 2 * ch + hp
                nc.tensor.matmul(
                    av[64 * w:64 * w + PP, ch, :],
                    ez[hp][0:113, w, ch, :],
                    v2[0:113, h, :],
                    tile_position=(0, 64 * w), start=True, stop=True)
    s[(g, "av")] = av_banks


def em_pe_ot(nc, ps, s, identb, g):
    out_tok = s.pop((g, "out_tok"))
    ot_flat = out_tok[:].rearrange("p ch hp d -> p (ch hp d)")
    ot_bank = ps.tile([128, 1024], BF16, tag="ps")
    ot_ps = ot_bank[:, 0:512].rearrange("p (nk t) -> p nk t", nk=4)
    for nk in range(4):
        nc.tensor.transpose(ot_ps[:, nk, 0:113],
                            ot_flat[0:113, 128 * nk:128 * nk + 128],
                            identb[0:113, 0:113])
    s[(g, "ot_ps")] = ot_ps


def em_pe_fin(nc, ps, s, wo_s, g):
    outT = s.pop((g, "outT"))
    fin_bank = ps.tile([128, 512], FP32, tag="ps")
    fin_ps = fin_bank[:, 0:DIM]
    for nk in range(4):
        nc.tensor.matmul(fin_ps[:], outT[:, nk, :], wo_s[:, nk, :],
                         start=(nk == 0), stop=(nk == 3))
    s[(g, "fin_ps")] = fin_ps


def em_act_ft(nc, sb, s, g):
    fT_ps = s.pop((g, "fT_ps"))
    fT = sb.tile([128, 2, 128], BF16, tag="fT")
    nc.scalar.copy(fT[:, 0, :], fT_ps[:, 0, :])
    nc.vector.tensor_copy(fT[:, 1, :], fT_ps[:, 1, :])
    s[(g, "fT")] = fT


def em_act_exp(nc, sb, s, ez, g):
    st_ps = s.pop((g, "st_ps"))
    for hp in range(2):
        for w in range(2):
            nc.scalar.activation(
                ez[hp][64 * w:64 * w + PP, w, :, :],
                st_ps[64 * hp:64 * hp + PP, w, :, :],
                mybir.ActivationFunctionType.Exp, scale=SCALE)
    s[(g, "expSz")] = ez


def em_act_outT(nc, sb, s, g):
    ot_ps = s.pop((g, "ot_ps"))
    outT = sb.tile([128, 4, 128], BF16, tag="outT")
    nc.vector.tensor_copy(outT[:], ot_ps[:])
    s[(g, "outT")] = outT


def em_dve_norm(nc, sb, s, g):
    av_banks = s.pop((g, "av"))
    out_tok = sb.tile([128, 4, 2, D], BF16, tag="out_tok")  # free = (ch, hp, d)
    for hp in range(2):
        av = av_banks[hp]
        recd = sb.tile([128, 4, 1], FP32, tag=f"recd{hp}")
        nc.vector.reciprocal(recd[0:113, :, :], av[0:113, :, D:D + 1])
        nc.vector.scalar_tensor_tensor(
            out=out_tok[0:113, :, hp, :],
            in0=av[0:113, :, 0:D],
            scalar=1.0,
            in1=recd[0:113, :, 0:1].broadcast_to((113, 4, D)),
            op0=mybir.AluOpType.mult, op1=mybir.AluOpType.mult)
    s[(g, "out_tok")] = out_tok


def em_dve_fin(nc, sb, s, bo_bc, g):
    fin_ps = s.pop((g, "fin_ps"))
    fin = sb.tile([128, DIM], FP32, tag="fin")
    nc.vector.scalar_tensor_tensor(out=fin[:], in0=fin_ps[:], scalar=1.0,
                                   in1=bo_bc[:], op0=mybir.AluOpType.mult,
                                   op1=mybir.AluOpType.add)
    s[(g, "fin")] = fin


def em_pe_qkv(nc, ps, s, wq_s, wk_s, wv_s, g):
    fT = s.pop((g, "fT"))
    fT_c = fT[:].rearrange("p kc (w ts) -> p kc w ts", w=2)[:, :, :, 0:PP]
    q_bank = ps.tile([128, 512], FP32, tag="ps")
    qT_ps = q_bank[:, 0:392].rearrange("p (nk w t) -> p nk w t", nk=4, w=2)
    k_bank = ps.tile([128, 512], FP32, tag="ps")
    kT_ps = k_bank[:, 0:392].rearrange("p (nk w t) -> p nk w t", nk=4, w=2)
    for nk in range(4):
        for kc in range(2):
            nc.tensor.matmul(qT_ps[:, nk, :, :],
                             wq_s[:, kc, 128 * nk:128 * nk + 128],
                             fT_c[:, kc, :, :], start=(kc == 0), stop=(kc == 1))
            nc.tensor.matmul(kT_ps[:, nk, :, :],
                             wk_s[:, kc, 128 * nk:128 * nk + 128],
                             fT_c[:, kc, :, :], start=(kc == 0), stop=(kc == 1))
    v_bank = ps.tile([128, 512], FP32, tag="ps")
    for kc in range(2):
        nc.tensor.matmul(v_bank[:], fT[:, kc, :], wv_s[:, kc, :],
                         start=(kc == 0), stop=(kc == 1))
    s[(g, "qT_ps")] = qT_ps
    s[(g, "kT_ps")] = kT_ps
    s[(g, "v_ps")] = v_bank


def em_copies_tail(nc, sb, s, kT2, v2, g):
    qT_ps = s.pop((g, "qT_ps"))
    kT_ps = s.pop((g, "kT_ps"))
    v_ps = s.pop((g, "v_ps"))

    qT2 = sb.tile([128, 4, 2, PP], BF16, tag="qT2")
    nc.vector.tensor_copy(qT2[:], qT_ps[:])
    nc.scalar.copy(
        kT2[0:64, :, :, 0:PP],
        kT_ps[0:64, :, :, :].rearrange("p nk w ts -> p w nk ts"))
    nc.vector.tensor_copy(
        kT2[64:128, :, :, 64:64 + PP],
        kT_ps[64:128, :, :, :].rearrange("p nk w ts -> p w nk ts"))
    nc.scalar.copy(v2[0:113, :, 0:D],
                   v_ps[0:113, :].rearrange("p (h d) -> p h d", h=H))
    s[(g, "qT2")] = qT2
    s[(g, "kT2")] = kT2
    s[(g, "v2")] = v2


def em_dma_out(nc, s, out, c, g):
    img, wx, u = c
    fin = s.pop((g, "fin"))
    for w, eng in ((0, nc.sync), (1, nc.gpsimd)):
        c0 = P * (2 * u + w)
        eng.dma_start(out=out[img, P * wx:P * wx + P, c0:c0 + P, :],
                      in_=fin[64 * w:64 * w + PP, :])


_CACHED = {}


def _get_nc():
    if "nc" not in _CACHED:
        _CACHED["nc"] = build_bass()
    return _CACHED["nc"]


def kernel(fmap, Wq, Wkv, Wo, bo, _trace=False, _trace_kwargs=None):
    import ml_dtypes
    fmap = np.ascontiguousarray(fmap).astype(ml_dtypes.bfloat16)
    nc = _get_nc()
    in_maps = []
    for c in range(NCORES):
        in_maps.append({
            "fmap": fmap[IMGS_PER_CORE * c:IMGS_PER_CORE * (c + 1)],
            "Wq": np.ascontiguousarray(Wq, dtype=np.float32),
            "Wkv": np.ascontiguousarray(Wkv, dtype=np.float32),
            "Wo": np.ascontiguousarray(Wo, dtype=np.float32),
            "bo": np.ascontiguousarray(bo, dtype=np.float32),
        })
    res = run_bass_kernel_spmd(nc, in_maps, core_ids=list(range(NCORES)),
                               trace=_trace, **(_trace_kwargs or {}))
    outs = [r["out"] for r in res.results]
    full = np.concatenate(outs, axis=0)
    if _trace:
        return full, res
    return full
